# revision 45
# baseline (speedup 1.0000x reference)
"""CMSA (cross-modal self-attention) Trainium2 Bass kernel.

Problem: two feature maps x,y of [B=4, C=256, H=64, W=64]. Per sample:
  q_y,k_y = 1x1conv(y) -> [32, N]; v_x = 1x1conv(x) -> [256, N]  (N=4096)
  att_y = softmax(q_y^T k_y); enhanced_x = v_x @ att_y^T + x
  (and symmetrically x->y). Output: (enhanced_x, enhanced_y).

Sharding: 8 independent attention problems = (4 samples) x (2 directions),
one per NeuronCore, SPMD. Per-core kernel computes one full attention.

Kernel math (per core):
  L^T[j,i] = sum_d k[d,j] q[d,i]     (k-tile stationary fp16, 2-way row-packed)
  U^T[j,i] = exp(L^T[j,i])           (unnormalized bf16; |logit|<~45 so exp
                                      stays well inside fp32/bf16 range)
  T[i, 0:256] = sum_j U^T[j,i] V^T[j,c]   "transposed AV": U^T-slice is the
  T[i, 256]   = sum_j U^T[j,i]             stationary operand, [V^T | ones]
                                           (257 cols) is the moving operand,
                                           so the denominator rides along as
                                           column 256 of the same psum tile
  out^T[i,c] = T[i,c] / T[i,256] + feat_v^T[i,c]

The output is written transposed ([N, C]) and flipped on the host.

Setup dataflow (engine assignment matters: the scalar engine's exp stream
and the PE are the walls; dtype casts ride for free on gpsimd-initiated
(SWDGE) DMAs, which convert f32->f16 in flight at ~190 GB/s):
  gpsimd queue: feature DMAs with in-flight f32->f16 cast (fqk first: the
                q/k projections gate the whole pipeline)
  SP queue:     weight/bias DMAs up front (tiny), output DMAs after
  PE:           p-state warmup; weight transposes; q/k/v projections (fp16
                operands, 1 col/cycle); fvT fp16 transposes; QK; AV
  DVE:          vTx/fvT psum drains, reciprocal, output chain
  Act:          q/k psum drains (Identity activation with per-partition
                bias) and exp
  bv is folded into the V^T projection as a rank-1 (ones-row x bv-row)
  psum accumulation step, so no partition-broadcast is needed.
AV groups are issued as half-groups (16 j-steps) between exps so PE
bursts stay short; the residual is fp16 (~5e-4 relative, well inside
the error budget).
"""

import numpy as np

import concourse.bass as bass
import concourse.tile as tile
from concourse import bacc, mybir
from concourse.bass_utils import run_bass_kernel_spmd
from concourse.masks import make_identity

C = 256
RD = 32
B = 4
N = 64 * 64  # 4096
NCORES = 8

IBLK = 512           # i-block size (query block)
NIB = N // IBLK      # 8
JT = 128             # j tile size
NJT = N // JT        # 32
ITPB = IBLK // 128   # 128-wide i-tiles per block = 4
VX = C + 1           # moving width of the AV matmul (values + ones column)

F32 = mybir.dt.float32
BF16 = mybir.dt.bfloat16
F16 = mybir.dt.float16


def _build_bass():
    nc = bacc.Bacc(
        "TRN2",
        target_bir_lowering=False,
        debug=False,
        num_devices=NCORES,
    )

    feat_qk = nc.dram_tensor("feat_qk", [C, N], F32, kind="ExternalInput").ap()
    feat_v = nc.dram_tensor("feat_v", [C, N], F32, kind="ExternalInput").ap()
    wq = nc.dram_tensor("wq", [RD, C], F32, kind="ExternalInput").ap()
    wk = nc.dram_tensor("wk", [RD, C], F32, kind="ExternalInput").ap()
    wv = nc.dram_tensor("wv", [C, C], F32, kind="ExternalInput").ap()
    bq = nc.dram_tensor("bq", [RD], F32, kind="ExternalInput").ap()
    bk = nc.dram_tensor("bk", [RD], F32, kind="ExternalInput").ap()
    bv = nc.dram_tensor("bv", [C], F32, kind="ExternalInput").ap()
    # transposed output [i, c]; host flips back to [C, N]
    out = nc.dram_tensor("out_t", [N, C], F32, kind="ExternalOutput").ap()

    with tile.TileContext(nc) as tc:
        _kernel_body(nc, tc, feat_qk, feat_v, wq, wk, wv, bq, bk, bv, out)
    nc.compile()
    return nc


def _kernel_body(nc, tc, feat_qk, feat_v, wq, wk, wv, bq, bk, bv, out):
    Exp = mybir.ActivationFunctionType.Exp
    with (
        tc.tile_pool(name="singles", bufs=1) as singles,
        tc.tile_pool(name="work", bufs=4) as work,
        tc.tile_pool(name="opool", bufs=6) as opool,
        tc.tile_pool(name="upool", bufs=32) as upool,
        tc.tile_pool(name="qk_psum", bufs=2, space="PSUM") as qk_psum,
        tc.tile_pool(name="av_psum", bufs=3, space="PSUM") as av_psum,
        tc.tile_pool(name="t_psum", bufs=1, space="PSUM") as t_psum,
    ):
        # PE warm-up: one accumulating matmul chain overlapping the input
        # DMA, so the clock ramps before real matmul work starts
        wu_w = singles.tile([128, 128], BF16, tag="wu_w")
        wu_x = singles.tile([128, 512], BF16, tag="wu_x")
        nc.vector.memset(wu_w, 1.0)
        nc.vector.memset(wu_x, 1.0)
        wup = av_psum.tile([128, 512], F32, tag="av")
        for w in range(8):
            nc.tensor.matmul(wup, wu_w, wu_x, start=(w == 0), stop=(w == 7))

        # fp16 feature tiles, written by casting SWDGE DMAs
        fqk16_sb = singles.tile([128, 2, N], F16, tag="fqk16")
        fv16_sb = singles.tile([128, 2, N], F16, tag="fv16")

        # projection outputs (live for the whole kernel)
        # q/k: fp16, rows replicated 2x (partitions 0-31 / 32-63) for
        # 2-way row packing of the QK matmuls
        q_sb = singles.tile([2 * RD, N], F16, tag="q")
        k_sb = singles.tile([2 * RD, N], F16, tag="k")
        # [V^T | ones] moving tiles: [j_inner, j_tile, VX] bf16 (col C = 1.0)
        vTx_sb = singles.tile([128, NJT, VX + 3], BF16, tag="vTx")
        nc.vector.memset(vTx_sb[:, :, C:], 1.0)
        # residual feat_v^T tiles: [i_inner, i_tile, c] fp16 (DMA XBAR)
        fvT_sb = singles.tile([128, N // 128, C], F16, tag="fvT")

        with tc.tile_pool(name="proj", bufs=1) as proj:
            identity = proj.tile([128, 128], F32, tag="identity")
            make_identity(nc, identity)

            # ---- load weights / biases (Act hwdge queue; tiny) ----
            wq_sb = proj.tile([RD, C], F32, tag="wq")
            wk_sb = proj.tile([RD, C], F32, tag="wk")
            wv_sb = proj.tile([128, 2, C], F32, tag="wv")  # wv rows chunked
            nc.sync.dma_start(out=wq_sb, in_=wq)
            nc.sync.dma_start(out=wk_sb, in_=wk)
            for co in range(2):
                nc.sync.dma_start(
                    out=wv_sb[:, co, :], in_=wv[co * 128 : (co + 1) * 128, :]
                )
            bq_sb = proj.tile([2 * RD, 1], F32, tag="bq")
            bk_sb = proj.tile([2 * RD, 1], F32, tag="bk")
            bv_sb = proj.tile([1, C], F32, tag="bv")
            for r in range(2):
                nc.sync.dma_start(
                    out=bq_sb[r * RD : (r + 1) * RD],
                    in_=bq.rearrange("(r o) -> r o", o=1),
                )
                nc.sync.dma_start(
                    out=bk_sb[r * RD : (r + 1) * RD],
                    in_=bk.rearrange("(r o) -> r o", o=1),
                )
            nc.sync.dma_start(out=bv_sb, in_=bv.rearrange("(o c) -> o c", o=1))

            # fp16 copies for the rank-1 bv accumulation in the V^T proj
            ones1_sb = proj.tile([1, JT], F16, tag="ones1")
            nc.vector.memset(ones1_sb, 1.0)
            bv16_sb = proj.tile([1, C], F16, tag="bv16")
            nc.vector.tensor_copy(out=bv16_sb, in_=bv_sb)

            # ---- transpose q/k/v weights (fp16 stationaries) ----
            wqT_sb = proj.tile([128, 2, 2 * RD], F16, tag="wqT")
            wkT_sb = proj.tile([128, 2, 2 * RD], F16, tag="wkT")
            for (w_sb, wT_sb) in ((wq_sb, wqT_sb), (wk_sb, wkT_sb)):
                for co in range(2):
                    tp = qk_psum.tile([128, RD], F32, tag="qk")
                    nc.tensor.transpose(
                        tp, w_sb[:, co * 128 : (co + 1) * 128], identity[:RD, :RD]
                    )
                    for r in range(2):
                        nc.vector.tensor_copy(
                            out=wT_sb[:, co, r * RD : (r + 1) * RD], in_=tp
                        )
            wvT_sb = proj.tile([128, 2, C], F16, tag="wvT")
            for o in range(2):
                for i in range(2):
                    tp = qk_psum.tile([128, 128], F32, tag="qk")
                    nc.tensor.transpose(
                        tp, wv_sb[:, o, i * 128 : (i + 1) * 128], identity
                    )
                    nc.vector.tensor_copy(
                        out=wvT_sb[:, i, o * 128 : (o + 1) * 128], in_=tp
                    )

            # ---- feat_qk stream: fast fp32 DMA (SP hwdge) -> DVE cast ->
            # q/k proj -> Act Identity-with-bias drain. (The casting SWDGE
            # queue runs at only ~190 GB/s, so it carries just feat_v,
            # concurrently with this chain.)
            Ident = mybir.ActivationFunctionType.Identity
            for nb in range(NIB):
                ns = bass.ts(nb, IBLK)
                for co in range(2):
                    nc.gpsimd.dma_start(
                        out=fqk16_sb[:, co, ns],
                        in_=feat_qk[co * 128 : (co + 1) * 128, ns],
                    )
                for (wT_sb, b_sb, dst) in (
                    (wqT_sb, bq_sb, q_sb),
                    (wkT_sb, bk_sb, k_sb),
                ):
                    pp = qk_psum.tile([2 * RD, IBLK], F32, tag="qk")
                    for co in range(2):
                        nc.tensor.matmul(
                            pp,
                            wT_sb[:, co, :],
                            fqk16_sb[:, co, ns],
                            start=(co == 0),
                            stop=(co == 1),
                        )
                    nc.scalar.activation(
                        out=dst[:, ns], in_=pp, func=Ident, bias=b_sb
                    )

            # ---- feat_v stream (trails fqk on the gpsimd queue) ----
            for nb in range(NIB):
                ns = bass.ts(nb, IBLK)
                for co in range(2):
                    nc.gpsimd.dma_start(
                        out=fv16_sb[:, co, ns],
                        in_=feat_v[co * 128 : (co + 1) * 128, ns],
                    )

            identity16 = proj.tile([128, 128], F16, tag="identity16")
            nc.vector.tensor_copy(out=identity16, in_=identity)

            # ---- block-0 filler units: one V^T-projection j-tile each ----
            def v_proj_unit(jt):
                def go():
                    vp = av_psum.tile([128, C], F32, tag="av")
                    for co in range(2):
                        nc.tensor.matmul(
                            vp, fv16_sb[:, co, bass.ts(jt, JT)], wvT_sb[:, co, :],
                            start=(co == 0), stop=False,
                        )
                    # rank-1 bv accumulation: vp[j, c] += 1 * bv[c]
                    nc.tensor.matmul(
                        vp, ones1_sb, bv16_sb, start=False, stop=True,
                    )
                    nc.vector.tensor_copy(out=vTx_sb[:, jt, 0:C], in_=vp)
                return go

            def fvt_quad(c, co):
                # 4 fp16 PE transposes of fv16 chunk c's co-half into one
                # psum tile, drained by a single 2x-mode DVE copy
                def go():
                    tp = t_psum.tile([128, ITPB, 128], F16, tag="qkt")
                    for sub in range(ITPB):
                        nc.tensor.transpose(
                            tp[:, sub, :],
                            fv16_sb[:, co, bass.ts(4 * c + sub, 128)],
                            identity16,
                        )
                    nc.vector.tensor_copy(
                        out=fvT_sb[:, 4 * c : 4 * c + 4,
                                   co * 128 : (co + 1) * 128],
                        in_=tp,
                    )
                return go

            # chunk-monotone interleave of v-proj and residual-transpose units
            fillers = []
            for c_ in range(NIB):
                fillers.append(fvt_quad(c_, 0))
                fillers.append(v_proj_unit(4 * c_))
                fillers.append(v_proj_unit(4 * c_ + 1))
                fillers.append(fvt_quad(c_, 1))
                fillers.append(v_proj_unit(4 * c_ + 2))
                fillers.append(v_proj_unit(4 * c_ + 3))

            _attention(nc, upool, work, opool, qk_psum, av_psum,
                       q_sb, k_sb, vTx_sb, fvT_sb, out, fillers, Exp)


def _attention(nc, upool, work, opool, qk_psum, av_psum,
               q_sb, k_sb, vTx_sb, fvT_sb, out, fillers, Exp):
    blocks = [(b * IBLK, IBLK) for b in range(NIB)]

    prev_u = None
    prev_blk = None

    pend_avt = {}

    def av_half(blk, it, half, u_list):
        # half of one 128-wide i-tile's AV accumulation (16 j-tiles), so the
        # PE bursts between exps stay short; on the second half the tile is
        # normalized, residual-added, and stored.
        # T[i, 0:C]=numer^T, T[i, C]=denominator
        start, _size = blk
        itg = start // 128 + it
        if half == 0:
            avt = av_psum.tile([128, VX + 3], F32, tag="av")
            pend_avt[it] = avt
        avt = pend_avt[it]
        for jt in range(half * NJT // 2, (half + 1) * NJT // 2):
            nc.tensor.matmul(
                avt[:, 0:VX],
                u_list[jt // 2][:, jt % 2, bass.ts(it, 128)],
                vTx_sb[:, jt, 0:VX],
                start=(jt == 0),
                stop=(jt == NJT - 1),
            )
        if half == 0:
            return
        recip = work.tile([128, 1], F32, tag="recip")
        nc.vector.reciprocal(recip, avt[:, C : C + 1])
        o = opool.tile([128, C], F32, tag="o")
        nc.vector.tensor_scalar(
            out=o, in0=avt[:, 0:C], scalar1=recip, scalar2=None,
            op0=mybir.AluOpType.mult,
        )
        nc.vector.tensor_add(o, o, fvT_sb[:, itg, :])
        nc.sync.dma_start(out=out[bass.ts(itg, 128), :], in_=o)

    def av_group(blk, it, u_list):
        av_half(blk, it, 0, u_list)
        av_half(blk, it, 1, u_list)

    for bi, blk in enumerate(blocks):
        bstart, bsize = blk
        ntiles_prev = prev_blk[1] // 128 if prev_blk else 0
        u_new = []
        for jp in range(NJT // 2):
            # two j tiles, 2-way row-packed (row groups 0 and 1), landing in
            # one psum tile -> a single exp
            lp = qk_psum.tile([JT, 2, IBLK], F32, tag="qk")
            for h in range(2):
                jt = 2 * jp + h
                rp = slice(RD * h, RD * (h + 1))
                nc.tensor.matmul(
                    lp[:, h, 0:bsize],
                    k_sb[rp, bass.ts(jt, JT)],
                    q_sb[rp, bstart : bstart + bsize],
                    start=True,
                    stop=True,
                    tile_position=(RD * h, 0),
                )
            ut = upool.tile([JT, 2, bsize], BF16, tag="u")
            nc.scalar.activation(out=ut, in_=lp[:, :, 0:bsize], func=Exp)
            u_new.append(ut)

            # interleaved trailing work: AV groups of the previous block,
            # or (block 0) the V^T-projection filler units
            if prev_u is not None:
                # 2 half-groups per step window -> shorter PE bursts
                nh = 2 * ntiles_prev
                step = (NJT // 2) // nh
                if jp % step == step - 1 and jp // step < nh:
                    av_half(prev_blk, (jp // step) // 2, (jp // step) % 2,
                            prev_u)
            else:
                for _ in range(3):
                    if fillers:
                        fillers.pop(0)()
        # any leftover fillers after block 0's steps
        if prev_u is None:
            while fillers:
                fillers.pop(0)()
        prev_u = u_new
        prev_blk = blk

    for it in range(prev_blk[1] // 128):
        av_group(prev_blk, it, prev_u)


_NC_CACHE = None


def _get_nc():
    global _NC_CACHE
    if _NC_CACHE is None:
        _NC_CACHE = _build_bass()
    return _NC_CACHE


def kernel(x_features, y_features, wqx, bqx, wkx, bkx, wvx, bvx,
           wqy, bqy, wky, bky, wvy, bvy):
    nc = _get_nc()

    def c(a):
        return np.ascontiguousarray(np.asarray(a), dtype=np.float32)

    in_maps = []
    for b in range(B):
        xf = c(x_features[b]).reshape(C, N)
        yf = c(y_features[b]).reshape(C, N)
        # core 2b: enhanced_x[b] — attention from y features, values from x
        in_maps.append({
            "feat_qk": yf, "feat_v": xf,
            "wq": c(wqy), "wk": c(wky), "wv": c(wvx),
            "bq": c(bqy), "bk": c(bky), "bv": c(bvx),
        })
        # core 2b+1: enhanced_y[b] — attention from x features, values from y
        in_maps.append({
            "feat_qk": xf, "feat_v": yf,
            "wq": c(wqx), "wk": c(wkx), "wv": c(wvy),
            "bq": c(bqx), "bk": c(bkx), "bv": c(bvy),
        })

    res = run_bass_kernel_spmd(nc, in_maps, core_ids=list(range(NCORES)))
    # out_t is [N, C]; flip back to [C, 64, 64]
    outs = [
        np.ascontiguousarray(r["out_t"].T).reshape(C, 64, 64)
        for r in res.results
    ]
    enhanced_x = np.stack(outs[0::2], axis=0)
    enhanced_y = np.stack(outs[1::2], axis=0)
    return enhanced_x, enhanced_y
